# revision 1
# baseline (speedup 1.0000x reference)
"""CodebookLoRASTELinear forward on 8 Trainium2 NeuronCores.

out = x @ (W_q + D)^T
  D   = (lora_B @ lora_A) * (alpha/rank)
  cb  = codebook / max|codebook|
  S   = exp(scale_log)                     (per [o, i//128] group)
  q   = cb[searchsorted(midpoints(cb), (W+D)/S)]
      == cb0 + sum_k d_k * ((W+D) > t_k*S)      (S > 0)
  W_q = q * S

Column-parallel sharding: W / scale / lora_B rows (out_features) are split
across the 8 cores; x and lora_A are replicated; per-core outputs are
concatenated on the host (no collectives).

Quantization runs in natural [o, i] layout (scale is a per-partition
scalar there; comparisons stay exact fp32 -- only the final big matmul is
f32r/TF32, which rounds operands to ~11 mantissa bits). Phase B is g-major
so the folded W_eff^T [128 (i%128), 32 (i//128), 512 (o)] fills
group-by-group and phase C's accumulation chains can start early. x tiles
are PE-transposed (f32r, 1.5 cyc/row) through PSUM and cast-copied into the
same folded layout; f32r matmuls (1 cyc/row) accumulate out[m(128), o(512)].
"""

import numpy as np
import sys

for _p in ("/opt/trn_rl_repo",):
    if _p not in sys.path:
        sys.path.insert(0, _p)

import concourse.mybir as mybir  # noqa: E402
import concourse.tile as tile  # noqa: E402
from concourse import bacc  # noqa: E402
from concourse.bass_utils import run_bass_kernel_spmd  # noqa: E402
from concourse.masks import make_identity  # noqa: E402
from contextlib import ExitStack  # noqa: E402

N_CORES = 8
M = 8192  # 4 * 2048 tokens
I = 4096  # in_features
O = 4096  # out_features
GROUP = 128
NG = I // GROUP  # 32 groups along i
RANK = 64
ALPHA_OVER_RANK = 32.0 / 64.0
OS = O // N_CORES  # 512 out features per core
NOB = OS // 128  # 4 output row blocks per core
NMB = M // 128  # 64 m blocks

F32 = mybir.dt.float32
F32R = mybir.dt.float32r

_cache = {}


def _build_program(cb0, tk, dk, reps=1):
    """cb0: smallest normalized codebook entry; tk: 3 bucket thresholds;
    dk: 3 successive codebook differences. All host floats baked in."""
    nc = bacc.Bacc("TRN2", target_bir_lowering=False, debug=False)

    x_d = nc.dram_tensor("x", [M, I], F32R, kind="ExternalInput").ap()
    w_d = nc.dram_tensor("w", [OS, I], F32, kind="ExternalInput").ap()
    scl_d = nc.dram_tensor("scl", [OS, NG], F32, kind="ExternalInput").ap()
    la_d = nc.dram_tensor("la", [RANK, I], F32, kind="ExternalInput").ap()
    lbt_d = nc.dram_tensor("lbt", [RANK, OS], F32, kind="ExternalInput").ap()
    out_d = nc.dram_tensor("out", [M, OS], F32, kind="ExternalOutput").ap()

    with tile.TileContext(nc) as tc, ExitStack() as ctx:
        singles = ctx.enter_context(tc.tile_pool(name="singles", bufs=1))

        ident = singles.tile([128, 128], F32)
        make_identity(nc, ident)
        identr = singles.tile([128, 128], F32R)
        nc.vector.tensor_copy(identr, ident)

        # per-partition scale scalars S = exp(scale_log), natural
        # [o%128, ob, g] layout ("scl" already holds exp values -- a
        # sub-ulp host exp keeps quantization decisions aligned with the
        # reference; the on-chip ACT Exp table is ~2e-6 off, which flips
        # buckets near thresholds)
        ssc = singles.tile([128, NOB, NG], F32)  # S
        for ob in range(NOB):
            nc.sync.dma_start(
                out=ssc[:, ob, :], in_=scl_d[ob * 128 : (ob + 1) * 128, :]
            )
        c0sc = singles.tile([128, NOB, NG], F32)  # cb0 * S
        nc.vector.tensor_scalar_mul(c0sc, ssc, float(cb0))
        tsc = []  # t_k * S
        for k in range(3):
            t = singles.tile([128, NOB, NG], F32, tag=f"tsc{k}")
            nc.vector.tensor_scalar_mul(t, ssc, float(tk[k]))
            tsc.append(t)

        la_sb = singles.tile([RANK, I], F32)
        nc.sync.dma_start(out=la_sb, in_=la_d)
        lbt_sb = singles.tile([RANK, OS], F32)
        nc.sync.dma_start(out=lbt_sb, in_=lbt_d)
        # fold alpha/rank into B^T once
        nc.vector.tensor_scalar_mul(lbt_sb, lbt_sb, float(ALPHA_OVER_RANK))

        # persistent effective transposed weight, folded [i%128, g, o], f32r
        weff = singles.tile([128, NG, OS], F32R)

        if reps > 1:
            ctx.enter_context(tc.For_i(0, reps, 1))

        # ---- phase B (g-major): lora + quantize -> transpose -> weff[g] ----
        wload = ctx.enter_context(tc.tile_pool(name="wload", bufs=8))
        qtmp = ctx.enter_context(tc.tile_pool(name="qtmp", bufs=3))
        wq = ctx.enter_context(tc.tile_pool(name="wq", bufs=8))
        psumD = ctx.enter_context(tc.tile_pool(name="psumD", bufs=2, space="PSUM"))
        psumW = ctx.enter_context(tc.tile_pool(name="psumW", bufs=2, space="PSUM"))

        for g in range(NG):
            gsl = slice(g * 128, (g + 1) * 128)
            # lora delta for all 4 o-blocks of this group, fp32-exact
            d_all = psumD.tile([128, NOB, 128], F32, tag="d")
            for ob in range(NOB):
                nc.tensor.matmul(
                    d_all[:, ob, :],
                    lhsT=lbt_sb[:, ob * 128 : (ob + 1) * 128],
                    rhs=la_sb[:, gsl],
                    start=True,
                    stop=True,
                )
            pt = psumW.tile([128, NOB, 128], F32R, tag="pt")
            for ob in range(NOB):
                wn = wload.tile([128, 128], F32, tag="wn")
                nc.sync.dma_start(out=wn, in_=w_d[ob * 128 : (ob + 1) * 128, gsl])
                u = qtmp.tile([128, 128], F32, tag="u")
                nc.vector.tensor_add(u, wn, d_all[:, ob, :])
                a1 = qtmp.tile([128, 128], F32, tag="a1")
                nc.vector.tensor_scalar(
                    a1, u, tsc[0][:, ob, g : g + 1], float(dk[0]),
                    op0=mybir.AluOpType.is_gt, op1=mybir.AluOpType.mult,
                )
                a2 = qtmp.tile([128, 128], F32, tag="a2")
                nc.vector.tensor_scalar(
                    a2, u, tsc[1][:, ob, g : g + 1], float(dk[1]),
                    op0=mybir.AluOpType.is_gt, op1=mybir.AluOpType.mult,
                )
                a3 = qtmp.tile([128, 128], F32, tag="a3")
                nc.vector.tensor_scalar(
                    a3, u, tsc[2][:, ob, g : g + 1], float(dk[2]),
                    op0=mybir.AluOpType.is_gt, op1=mybir.AluOpType.mult,
                )
                # staircase sum on the (otherwise idle) gpsimd engine
                nc.gpsimd.tensor_add(a1, a1, a2)
                nc.gpsimd.tensor_add(a1, a1, a3)
                # m = (q - cb0)*S + cb0*S
                nc.vector.tensor_scalar(
                    a1, a1, ssc[:, ob, g : g + 1], c0sc[:, ob, g : g + 1],
                    op0=mybir.AluOpType.mult, op1=mybir.AluOpType.add,
                )
                # w_eff = q*S + D, rounded to f32r on write
                wqn = wq.tile([128, 128], F32R, tag="wq")
                nc.vector.tensor_add(wqn, a1, d_all[:, ob, :])
                nc.tensor.transpose(pt[:, ob, :], wqn, identr)
            # one cast-copy lands the whole group row of W_eff^T
            nc.scalar.copy(weff[:, g, :], pt.bitcast(F32))

        # ---- phase C: stream x, transpose, matmul --------------------------
        xpool = ctx.enter_context(tc.tile_pool(name="xpool", bufs=2))
        xtpool = ctx.enter_context(tc.tile_pool(name="xtpool", bufs=3))
        opool = ctx.enter_context(tc.tile_pool(name="opool", bufs=3))
        psumT = ctx.enter_context(tc.tile_pool(name="psumT", bufs=2, space="PSUM"))
        psumO = ctx.enter_context(tc.tile_pool(name="psumO", bufs=2, space="PSUM"))

        for mb in range(NMB):
            x_t = xpool.tile([128, I], F32R, tag="x")
            nc.sync.dma_start(out=x_t, in_=x_d[mb * 128 : (mb + 1) * 128, :])

            xT = xtpool.tile([128, NG, 128], F32R, tag="xT")
            for q in range(NG // 4):
                pxt = psumT.tile([128, 4, 128], F32R, tag="pxt")
                for j in range(4):
                    g = q * 4 + j
                    nc.tensor.transpose(
                        pxt[:, j, :], x_t[:, g * 128 : (g + 1) * 128], identr
                    )
                # cast-copy (bitcast input so the verifier sees an f32->f32r
                # rounding op; transpose output doesn't count as rounded)
                if q % 2 == 0:
                    nc.scalar.copy(xT[:, q * 4 : (q + 1) * 4, :], pxt.bitcast(F32))
                else:
                    nc.vector.tensor_copy(xT[:, q * 4 : (q + 1) * 4, :],
                                          pxt.bitcast(F32))

            p_out = psumO.tile([128, OS], F32, tag="p_out")
            for g in range(NG):
                nc.tensor.matmul(
                    p_out,
                    lhsT=xT[:, g, :],
                    rhs=weff[:, g, :],
                    start=(g == 0),
                    stop=(g == NG - 1),
                )

            o_sb = opool.tile([128, OS], F32, tag="o")
            nc.scalar.copy(o_sb, p_out)
            nc.sync.dma_start(out=out_d[mb * 128 : (mb + 1) * 128, :], in_=o_sb)

    nc.compile()
    return nc


def _get_program(cb0, tk, dk, reps=1):
    key = (round(float(cb0), 9), tuple(round(float(t), 9) for t in tk),
           tuple(round(float(d), 9) for d in dk), reps)
    if key not in _cache:
        _cache[key] = _build_program(cb0, tk, dk, reps)
    return _cache[key]


def kernel(x, weight, scale_log, codebook, lora_A, lora_B):
    xf = np.ascontiguousarray(x.reshape(M, I), dtype=np.float32)

    cb = np.asarray(codebook, dtype=np.float64)
    cb = cb / max(float(np.max(np.abs(cb))), 1e-8)
    tk = (cb[:-1] + cb[1:]) * 0.5
    dk = np.diff(cb)

    nc = _get_program(float(cb[0]), [float(v) for v in tk], [float(v) for v in dk])

    in_maps = []
    for c in range(N_CORES):
        sl = slice(c * OS, (c + 1) * OS)
        in_maps.append({
            "x": xf,
            "w": np.ascontiguousarray(weight[sl], dtype=np.float32),
            "scl": np.exp(np.ascontiguousarray(
                scale_log.reshape(O, NG)[sl], dtype=np.float32)),
            "la": np.ascontiguousarray(lora_A, dtype=np.float32),
            "lbt": np.ascontiguousarray(lora_B[sl].T, dtype=np.float32),
        })

    res = run_bass_kernel_spmd(nc, in_maps, core_ids=list(range(N_CORES))).results
    out = np.concatenate([res[c]["out"] for c in range(N_CORES)], axis=1)
    return out.reshape(x.shape[0], x.shape[1], O)



# revision 9
# speedup vs baseline: 1.0892x; 1.0892x over previous
"""CodebookLoRASTELinear forward on 8 Trainium2 NeuronCores.

out = x @ (W_q + D)^T
  D   = (lora_B @ lora_A) * (alpha/rank)
  cb  = codebook / max|codebook|
  S   = exp(scale_log)                     (per [o, i//128] group)
  q   = cb[searchsorted(midpoints(cb), (W+D)/S)]
      == cb0 + sum_k d_k * ((W+D) > t_k*S)      (S > 0)
  W_q = q * S

Column-parallel sharding: W / scale / lora_B rows (out_features) are split
across the 8 cores; x and lora_A are replicated; per-core outputs are
concatenated on the host (no collectives).

x is pre-transposed and packed on the host into [i%128, g, m] chunk order
(bf16), so phase C needs no on-chip transposes: the big matmul streams
out[m(128), o(512)] tiles directly from xT chunks (lhsT) against the
folded W_eff^T [128 (i%128), g, 512 (o)] (rhs), both bf16 (fp32 PSUM
accumulation; rel-err budget ~1.6e-3 vs the 2e-2 gate).

Quantization (phase B) runs on-device in natural [o, i] layout where the
per-(o, group) scale is a per-partition scalar; compare inputs stay exact
fp32 (host exp for S, full-fp32 PE matmul for D) so bucket decisions match
the reference. The elementwise chain is spread across ACT / GpSimd / DVE
with fused scalar_tensor_tensor ops, and each finished W_eff^T group is
immediately consumed by the first output chunk's matmul chains so the PE
stays busy during phase B.
"""

import numpy as np
import sys

for _p in ("/opt/trn_rl_repo",):
    if _p not in sys.path:
        sys.path.insert(0, _p)

import ml_dtypes  # noqa: E402
import concourse.mybir as mybir  # noqa: E402
import concourse.tile as tile  # noqa: E402
from concourse import bacc  # noqa: E402
from concourse.bass_utils import run_bass_kernel_spmd  # noqa: E402
from concourse.masks import make_identity  # noqa: E402
from contextlib import ExitStack  # noqa: E402

N_CORES = 8
M = 8192  # 4 * 2048 tokens
I = 4096  # in_features
O = 4096  # out_features
GROUP = 128
NG = I // GROUP  # 32 groups along i
RANK = 64
ALPHA_OVER_RANK = 32.0 / 64.0
OS = O // N_CORES  # 512 out features per core
NOB = OS // 128  # 4 output row blocks per core
MSB = 512  # m columns per x chunk
NMSB = M // MSB  # 16 chunks
NMB = M // 128  # 64 output row blocks

F32 = mybir.dt.float32
F32R = mybir.dt.float32r
BF16 = mybir.dt.bfloat16
ALU = mybir.AluOpType

_cache = {}


def _build_program(cb0, tk, dk, reps=1):
    """cb0: smallest normalized codebook entry; tk: 3 bucket thresholds;
    dk: 3 successive codebook differences. All host floats baked in."""
    nc = bacc.Bacc("TRN2", target_bir_lowering=False, debug=False)

    xt_d = nc.dram_tensor("xt", [NMSB * 128, NG, MSB], BF16,
                          kind="ExternalInput").ap()
    w_d = nc.dram_tensor("w", [OS, NG, GROUP], F32, kind="ExternalInput").ap()
    scl_d = nc.dram_tensor("scl", [OS, NG], F32, kind="ExternalInput").ap()
    la_d = nc.dram_tensor("la", [RANK, I], F32, kind="ExternalInput").ap()
    lbt_d = nc.dram_tensor("lbt", [RANK, OS], F32, kind="ExternalInput").ap()
    out_d = nc.dram_tensor("out", [M, OS], F32, kind="ExternalOutput").ap()

    with tile.TileContext(nc) as tc, ExitStack() as ctx:
        singles = ctx.enter_context(tc.tile_pool(name="singles", bufs=1))

        ident = singles.tile([128, 128], F32)
        make_identity(nc, ident)
        identr = singles.tile([128, 128], F32R)
        nc.vector.tensor_copy(identr, ident)

        # per-partition scale scalars S = exp(scale_log), natural
        # [o%128, ob, g] layout ("scl" already holds exp values -- a
        # sub-ulp host exp keeps quantization decisions aligned with the
        # reference; the on-chip ACT Exp table is ~2e-6 off, which flips
        # buckets near thresholds)
        ssc = singles.tile([128, NOB, NG], F32)  # S
        for ob in range(NOB):
            nc.sync.dma_start(
                out=ssc[:, ob, :], in_=scl_d[ob * 128 : (ob + 1) * 128, :]
            )
        c0sc = singles.tile([128, NOB, NG], F32)  # cb0 * S
        nc.vector.tensor_scalar_mul(c0sc, ssc, float(cb0))
        # shifted thresholds (t_k + cb0) * S: phase B compares
        # v = W + D + cb0*S against these, avoiding any subtract ops
        tsc = []
        for k in range(3):
            t = singles.tile([128, NOB, NG], F32, tag=f"tsc{k}")
            nc.vector.tensor_scalar_mul(t, ssc, float(tk[k] + cb0))
            tsc.append(t)

        la_sb = singles.tile([RANK, I], F32)
        nc.sync.dma_start(out=la_sb, in_=la_d)
        lbt_sb = singles.tile([RANK, OS], F32)
        nc.sync.dma_start(out=lbt_sb, in_=lbt_d)
        # fold alpha/rank into B^T once
        nc.vector.tensor_scalar_mul(lbt_sb, lbt_sb, float(ALPHA_OVER_RANK))

        # persistent effective transposed weight, one tile per group so
        # phase C matmuls can consume groups as phase B finishes them
        weff = [
            singles.tile([128, OS], BF16, tag=f"weff{g}", name=f"weff{g}")
            for g in range(NG)
        ]

        if reps > 1:
            ctx.enter_context(tc.For_i(0, reps, 1))

        xpool = ctx.enter_context(tc.tile_pool(name="xpool", bufs=2))
        wload = ctx.enter_context(tc.tile_pool(name="wload", bufs=2))
        dpool = ctx.enter_context(tc.tile_pool(name="dpool", bufs=4))
        upool = ctx.enter_context(tc.tile_pool(name="upool", bufs=4))
        a1p = ctx.enter_context(tc.tile_pool(name="a1p", bufs=4))
        a2p = ctx.enter_context(tc.tile_pool(name="a2p", bufs=4))
        a12p = ctx.enter_context(tc.tile_pool(name="a12p", bufs=4))
        a123p = ctx.enter_context(tc.tile_pool(name="a123p", bufs=4))
        wqp = ctx.enter_context(tc.tile_pool(name="wqp", bufs=4))
        opool = ctx.enter_context(tc.tile_pool(name="opool", bufs=4))
        psumD = ctx.enter_context(tc.tile_pool(name="psumD", bufs=2, space="PSUM"))
        psumW = ctx.enter_context(tc.tile_pool(name="psumW", bufs=2, space="PSUM"))
        psumO = ctx.enter_context(tc.tile_pool(name="psumO", bufs=1, space="PSUM"))

        def load_chunk(msb):
            t = xpool.tile([128, NG, MSB], BF16, tag="xt")
            nc.sync.dma_start(out=t, in_=xt_d[msb * 128 : (msb + 1) * 128, :, :])
            return t

        NGQ = NG // 4  # W arrives in quads of groups

        def load_wquad(gq):
            ts = []
            for ob in range(NOB):
                t = wload.tile([128, 4, GROUP], F32, tag=f"w{ob}")
                nc.sync.dma_start(
                    out=t,
                    in_=w_d[ob * 128 : (ob + 1) * 128, gq * 4 : (gq + 1) * 4, :],
                )
                ts.append(t)
            return ts

        xt0 = load_chunk(0)
        xt1 = load_chunk(1)
        wq_tiles = {0: load_wquad(0), 1: load_wquad(1)}

        # output accumulation chains for chunk 0, fed group-by-group as
        # phase B completes each W_eff^T group
        chains0 = [
            psumO.tile([128, OS], F32, tag=f"o{mb}", name=f"chain{mb}")
            for mb in range(NOB)
        ]

        d3_is_one = abs(float(dk[2]) - 1.0) < 1e-12

        # ---- phase B: lora + quantize -> transpose -> weff[g], with the
        # ---- first chunk's matmuls interleaved per finished group ------
        for g in range(NG):
            gq, gi = divmod(g, 4)
            if gi == 0 and gq + 2 < NGQ:
                wq_tiles[gq + 2] = load_wquad(gq + 2)
            wg = weff[g]
            for ob in range(NOB):
                sS = ssc[:, ob, g : g + 1]
                sC0 = c0sc[:, ob, g : g + 1]
                wn = wq_tiles[gq][ob][:, gi, :]
                # lora delta, fp32-exact
                d_ps = psumD.tile([128, 128], F32, tag="d")
                nc.tensor.matmul(
                    d_ps,
                    lhsT=lbt_sb[:, ob * 128 : (ob + 1) * 128],
                    rhs=la_sb[:, g * 128 : (g + 1) * 128],
                    start=True,
                    stop=True,
                )
                # d_sb = D + cb0*S (bias folded in during the PSUM read)
                d_sb = dpool.tile([128, 128], F32, tag="dsb")
                nc.scalar.activation(
                    d_sb, d_ps, mybir.ActivationFunctionType.Identity,
                    bias=sC0, scale=1.0,
                )
                # v = W + D + cb0*S, compared against (t_k - cb0)*S below
                u = upool.tile([128, 128], F32, tag="u")
                nc.gpsimd.tensor_add(u, wn, d_sb)
                a1 = a1p.tile([128, 128], F32, tag="a1")
                nc.vector.tensor_scalar(
                    a1, u, tsc[0][:, ob, g : g + 1], float(dk[0]),
                    op0=ALU.is_gt, op1=ALU.mult,
                )
                a2 = a2p.tile([128, 128], F32, tag="a2")
                nc.vector.tensor_scalar(
                    a2, u, tsc[1][:, ob, g : g + 1], float(dk[1]),
                    op0=ALU.is_gt, op1=ALU.mult,
                )
                a12 = a12p.tile([128, 128], F32, tag="a12")
                nc.gpsimd.tensor_add(a12, a1, a2)
                a123 = a123p.tile([128, 128], F32, tag="a123")
                if d3_is_one:
                    nc.vector.scalar_tensor_tensor(
                        a123, u, tsc[2][:, ob, g : g + 1], a12,
                        op0=ALU.is_gt, op1=ALU.add,
                    )
                else:
                    a3 = a123p.tile([128, 128], F32, tag="a3")
                    nc.vector.tensor_scalar(
                        a3, u, tsc[2][:, ob, g : g + 1], float(dk[2]),
                        op0=ALU.is_gt, op1=ALU.mult,
                    )
                    nc.gpsimd.tensor_add(a123, a12, a3)
                # w_eff = (q - cb0)*S + (D + cb0*S) = q*S + D
                wq_t = wqp.tile([128, 128], F32R, tag="wq")
                nc.vector.scalar_tensor_tensor(
                    wq_t, a123, sS, d_sb, op0=ALU.mult, op1=ALU.add
                )
                pt = psumW.tile([128, 128], F32R, tag="pt")
                nc.tensor.transpose(pt, wq_t, identr)
                # cast-copy lands this ob's W_eff^T column block as bf16
                nc.scalar.copy(wg[:, ob * 128 : (ob + 1) * 128], pt.bitcast(F32))
            # chunk 0 consumes the finished group immediately
            for mb in range(NOB):
                nc.tensor.matmul(
                    chains0[mb],
                    lhsT=xt0[:, g, mb * 128 : (mb + 1) * 128],
                    rhs=wg,
                    start=(g == 0),
                    stop=(g == NG - 1),
                    skip_group_check=True,
                )

        for mb in range(NOB):
            o_sb = opool.tile([128, OS], F32, tag="o")
            nc.scalar.copy(o_sb, chains0[mb])
            nc.sync.dma_start(out=out_d[mb * 128 : (mb + 1) * 128, :], in_=o_sb)

        # ---- phase C: stream remaining x chunks through the big matmul ----
        xts = {0: xt0, 1: xt1}
        for msb in range(1, NMSB):
            if msb + 1 < NMSB:
                xts[msb + 1] = load_chunk(msb + 1)
            xt_t = xts.pop(msb)
            for mb in range(NOB):
                mbg = msb * NOB + mb
                p_out = psumO.tile([128, OS], F32, tag=f"o{mb}")
                for g in range(NG):
                    nc.tensor.matmul(
                        p_out,
                        lhsT=xt_t[:, g, mb * 128 : (mb + 1) * 128],
                        rhs=weff[g],
                        start=(g == 0),
                        stop=(g == NG - 1),
                    )
                o_sb = opool.tile([128, OS], F32, tag="o")
                nc.scalar.copy(o_sb, p_out)
                nc.sync.dma_start(
                    out=out_d[mbg * 128 : (mbg + 1) * 128, :], in_=o_sb
                )

    nc.compile()
    return nc


def _get_program(cb0, tk, dk, reps=1):
    key = (round(float(cb0), 9), tuple(round(float(t), 9) for t in tk),
           tuple(round(float(d), 9) for d in dk), reps)
    if key not in _cache:
        _cache[key] = _build_program(cb0, tk, dk, reps)
    return _cache[key]


def _codebook_consts(codebook):
    cb = np.asarray(codebook, dtype=np.float64)
    cb = cb / max(float(np.max(np.abs(cb))), 1e-8)
    tk = (cb[:-1] + cb[1:]) * 0.5
    dk = np.diff(cb)
    return float(cb[0]), [float(v) for v in tk], [float(v) for v in dk]


def _prep_in_maps(x, weight, scale_log, lora_A, lora_B):
    xf = np.ascontiguousarray(x.reshape(M, I), dtype=np.float32)
    # pack x^T chunks: xt[msb*128 + p, g, m'] = x[msb*MSB + m', g*128 + p]
    xt = (
        xf.reshape(NMSB, MSB, NG, GROUP)
        .transpose(0, 3, 2, 1)
        .astype(ml_dtypes.bfloat16)
        .reshape(NMSB * 128, NG, MSB)
    )
    in_maps = []
    for c in range(N_CORES):
        sl = slice(c * OS, (c + 1) * OS)
        in_maps.append({
            "xt": xt,
            "w": np.ascontiguousarray(
                weight[sl], dtype=np.float32).reshape(OS, NG, GROUP),
            "scl": np.exp(np.ascontiguousarray(
                scale_log.reshape(O, NG)[sl], dtype=np.float32)),
            "la": np.ascontiguousarray(lora_A, dtype=np.float32),
            "lbt": np.ascontiguousarray(lora_B[sl].T, dtype=np.float32),
        })
    return in_maps


def kernel(x, weight, scale_log, codebook, lora_A, lora_B):
    cb0, tk, dk = _codebook_consts(codebook)
    nc = _get_program(cb0, tk, dk)
    in_maps = _prep_in_maps(x, weight, scale_log, lora_A, lora_B)
    res = run_bass_kernel_spmd(nc, in_maps, core_ids=list(range(N_CORES))).results
    out = np.concatenate([res[c]["out"] for c in range(N_CORES)], axis=1)
    return out.reshape(x.shape[0], x.shape[1], O)


# revision 10
# speedup vs baseline: 1.4986x; 1.3758x over previous
"""CodebookLoRASTELinear forward on 8 Trainium2 NeuronCores.

out = x @ (W_q + D)^T
  D   = (lora_B @ lora_A) * (alpha/rank)
  cb  = codebook / max|codebook|
  S   = exp(scale_log)                     (per [o, i//128] group)
  q   = cb[searchsorted(midpoints(cb), (W+D)/S)]
      == cb0 + sum_k d_k * ((W+D) > t_k*S)      (S > 0)
  W_q = q * S

Column-parallel sharding: W / scale / lora_B rows (out_features) are split
across the 8 cores; x and lora_A are replicated; per-core outputs are
concatenated on the host (no collectives).

x is pre-transposed and packed on the host into [i%128, g, m] chunk order
(bf16), so phase C needs no on-chip transposes: the big matmul streams
out[m(128), o(512)] tiles directly from xT chunks (lhsT) against the
folded W_eff^T [128 (i%128), g, 512 (o)] (rhs), both bf16 (fp32 PSUM
accumulation; rel-err budget ~1.6e-3 vs the 2e-2 gate).

Quantization (phase B) runs on-device in natural [o, i] layout where the
per-(o, group) scale is a per-partition scalar; compare inputs stay exact
fp32 (host exp for S, full-fp32 PE matmul for D) so bucket decisions match
the reference. The elementwise chain is spread across ACT / GpSimd / DVE
with fused scalar_tensor_tensor ops, and each finished W_eff^T group is
immediately consumed by the first output chunk's matmul chains so the PE
stays busy during phase B.
"""

import numpy as np
import sys

for _p in ("/opt/trn_rl_repo",):
    if _p not in sys.path:
        sys.path.insert(0, _p)

import ml_dtypes  # noqa: E402
import concourse.mybir as mybir  # noqa: E402
import concourse.tile as tile  # noqa: E402
from concourse import bacc  # noqa: E402
from concourse.bass_utils import run_bass_kernel_spmd  # noqa: E402
from concourse.masks import make_identity  # noqa: E402
from contextlib import ExitStack  # noqa: E402

N_CORES = 8
M = 8192  # 4 * 2048 tokens
I = 4096  # in_features
O = 4096  # out_features
GROUP = 128
NG = I // GROUP  # 32 groups along i
RANK = 64
ALPHA_OVER_RANK = 32.0 / 64.0
OS = O // N_CORES  # 512 out features per core
NOB = OS // 128  # 4 output row blocks per core
MSB = 512  # m columns per x chunk
NMSB = M // MSB  # 16 chunks
NMB = M // 128  # 64 output row blocks

F32 = mybir.dt.float32
F32R = mybir.dt.float32r
BF16 = mybir.dt.bfloat16
ALU = mybir.AluOpType

_cache = {}


def _build_program(cb0, tk, dk, reps=1):
    """cb0: smallest normalized codebook entry; tk: 3 bucket thresholds;
    dk: 3 successive codebook differences. All host floats baked in."""
    nc = bacc.Bacc("TRN2", target_bir_lowering=False, debug=False)

    xt_d = nc.dram_tensor("xt", [NMSB * 128, NG, MSB], BF16,
                          kind="ExternalInput").ap()
    w_d = nc.dram_tensor("w", [OS, NG, GROUP], F32, kind="ExternalInput").ap()
    scl_d = nc.dram_tensor("scl", [OS, NG], F32, kind="ExternalInput").ap()
    la_d = nc.dram_tensor("la", [RANK, I], F32, kind="ExternalInput").ap()
    lbt_d = nc.dram_tensor("lbt", [RANK, OS], F32, kind="ExternalInput").ap()
    out_d = nc.dram_tensor("out", [M, OS], F32, kind="ExternalOutput").ap()

    with tile.TileContext(nc) as tc, ExitStack() as ctx:
        singles = ctx.enter_context(tc.tile_pool(name="singles", bufs=1))

        ident = singles.tile([128, 128], F32)
        make_identity(nc, ident)
        identr = singles.tile([128, 128], F32R)
        nc.vector.tensor_copy(identr, ident)

        # per-partition scale scalars S = exp(scale_log), natural
        # [o%128, ob, g] layout ("scl" already holds exp values -- a
        # sub-ulp host exp keeps quantization decisions aligned with the
        # reference; the on-chip ACT Exp table is ~2e-6 off, which flips
        # buckets near thresholds)
        ssc = singles.tile([128, NOB, NG], F32)  # S
        for ob in range(NOB):
            nc.sync.dma_start(
                out=ssc[:, ob, :], in_=scl_d[ob * 128 : (ob + 1) * 128, :]
            )
        c0sc = singles.tile([128, NOB, NG], F32)  # cb0 * S
        nc.vector.tensor_scalar_mul(c0sc, ssc, float(cb0))
        # shifted thresholds (t_k + cb0) * S: phase B compares
        # v = W + D + cb0*S against these, avoiding any subtract ops
        tsc = []
        for k in range(3):
            t = singles.tile([128, NOB, NG], F32, tag=f"tsc{k}")
            nc.vector.tensor_scalar_mul(t, ssc, float(tk[k] + cb0))
            tsc.append(t)

        la_sb = singles.tile([RANK, I], F32)
        nc.sync.dma_start(out=la_sb, in_=la_d)
        lbt_sb = singles.tile([RANK, OS], F32)
        nc.sync.dma_start(out=lbt_sb, in_=lbt_d)
        # fold alpha/rank into B^T once
        nc.vector.tensor_scalar_mul(lbt_sb, lbt_sb, float(ALPHA_OVER_RANK))

        # persistent effective transposed weight, one tile per group so
        # phase C matmuls can consume groups as phase B finishes them
        weff = [
            singles.tile([128, OS], BF16, tag=f"weff{g}", name=f"weff{g}")
            for g in range(NG)
        ]

        if reps > 1:
            ctx.enter_context(tc.For_i(0, reps, 1))

        xpool = ctx.enter_context(tc.tile_pool(name="xpool", bufs=2))
        wload = ctx.enter_context(tc.tile_pool(name="wload", bufs=2))
        dpool = ctx.enter_context(tc.tile_pool(name="dpool", bufs=4))
        upool = ctx.enter_context(tc.tile_pool(name="upool", bufs=4))
        a1p = ctx.enter_context(tc.tile_pool(name="a1p", bufs=4))
        a2p = ctx.enter_context(tc.tile_pool(name="a2p", bufs=4))
        a12p = ctx.enter_context(tc.tile_pool(name="a12p", bufs=4))
        a123p = ctx.enter_context(tc.tile_pool(name="a123p", bufs=4))
        wqp = ctx.enter_context(tc.tile_pool(name="wqp", bufs=8))
        opool = ctx.enter_context(tc.tile_pool(name="opool", bufs=4))
        psumD = ctx.enter_context(tc.tile_pool(name="psumD", bufs=2, space="PSUM"))
        psumW = ctx.enter_context(tc.tile_pool(name="psumW", bufs=2, space="PSUM"))
        psumO = ctx.enter_context(tc.tile_pool(name="psumO", bufs=1, space="PSUM"))

        def load_chunk(msb):
            t = xpool.tile([128, NG, MSB], BF16, tag="xt")
            nc.sync.dma_start(out=t, in_=xt_d[msb * 128 : (msb + 1) * 128, :, :])
            return t

        NGQ = NG // 4  # W arrives in quads of groups

        def load_wquad(gq):
            ts = []
            for ob in range(NOB):
                t = wload.tile([128, 4, GROUP], F32, tag=f"w{ob}")
                nc.sync.dma_start(
                    out=t,
                    in_=w_d[ob * 128 : (ob + 1) * 128, gq * 4 : (gq + 1) * 4, :],
                )
                ts.append(t)
            return ts

        xt0 = load_chunk(0)
        xt1 = load_chunk(1)
        wq_tiles = {0: load_wquad(0), 1: load_wquad(1)}

        # output accumulation chains for chunk 0, fed group-by-group as
        # phase B completes each W_eff^T group
        chains0 = [
            psumO.tile([128, OS], F32, tag=f"o{mb}", name=f"chain{mb}")
            for mb in range(NOB)
        ]

        d3_is_one = abs(float(dk[2]) - 1.0) < 1e-12

        # ---- phase B: lora + quantize -> transpose -> weff[g], with the
        # first chunk's matmuls interleaved per finished group. The PE
        # stream is software-pipelined: lora matmuls for group g go out
        # with chunk-0 matmuls for g-2 and transposes for g-1, so the
        # ACT/GpSimd/DVE elementwise chain between a group's lora matmul
        # and its transpose has a full PE round to complete (otherwise
        # the in-order PE queue stalls on every tile).
        wq_sb = {}  # g -> quantized+lora'd natural-layout tiles, per ob

        def emit_lora(g):
            d_ps = psumD.tile([128, NOB, 128], F32, tag="d", name=f"dall{g}")
            for ob in range(NOB):
                nc.tensor.matmul(
                    d_ps[:, ob, :],
                    lhsT=lbt_sb[:, ob * 128 : (ob + 1) * 128],
                    rhs=la_sb[:, g * 128 : (g + 1) * 128],
                    start=True,
                    stop=True,
                )
            return d_ps

        def emit_elemwise(g, d_ps):
            gq, gi = divmod(g, 4)
            tiles = []
            for ob in range(NOB):
                sS = ssc[:, ob, g : g + 1]
                sC0 = c0sc[:, ob, g : g + 1]
                wn = wq_tiles[gq][ob][:, gi, :]
                # d_sb = D + cb0*S (bias folded in during the PSUM read)
                d_sb = dpool.tile([128, 128], F32, tag="dsb")
                nc.scalar.activation(
                    d_sb, d_ps[:, ob, :], mybir.ActivationFunctionType.Identity,
                    bias=sC0, scale=1.0,
                )
                # v = W + D + cb0*S, compared against (t_k + cb0)*S below
                u = upool.tile([128, 128], F32, tag="u")
                nc.gpsimd.tensor_add(u, wn, d_sb)
                a1 = a1p.tile([128, 128], F32, tag="a1")
                nc.vector.tensor_scalar(
                    a1, u, tsc[0][:, ob, g : g + 1], float(dk[0]),
                    op0=ALU.is_gt, op1=ALU.mult,
                )
                a2 = a2p.tile([128, 128], F32, tag="a2")
                nc.vector.tensor_scalar(
                    a2, u, tsc[1][:, ob, g : g + 1], float(dk[1]),
                    op0=ALU.is_gt, op1=ALU.mult,
                )
                a12 = a12p.tile([128, 128], F32, tag="a12")
                nc.gpsimd.tensor_add(a12, a1, a2)
                a123 = a123p.tile([128, 128], F32, tag="a123")
                if d3_is_one:
                    nc.vector.scalar_tensor_tensor(
                        a123, u, tsc[2][:, ob, g : g + 1], a12,
                        op0=ALU.is_gt, op1=ALU.add,
                    )
                else:
                    a3 = a123p.tile([128, 128], F32, tag="a3")
                    nc.vector.tensor_scalar(
                        a3, u, tsc[2][:, ob, g : g + 1], float(dk[2]),
                        op0=ALU.is_gt, op1=ALU.mult,
                    )
                    nc.gpsimd.tensor_add(a123, a12, a3)
                # w_eff = (q - cb0)*S + (D + cb0*S) = q*S + D
                wq_t = wqp.tile([128, 128], F32R, tag="wq")
                nc.vector.scalar_tensor_tensor(
                    wq_t, a123, sS, d_sb, op0=ALU.mult, op1=ALU.add
                )
                tiles.append(wq_t)
            wq_sb[g] = tiles

        def emit_transpose(g):
            wg = weff[g]
            for ob in range(NOB):
                pt = psumW.tile([128, 128], F32R, tag="pt")
                nc.tensor.transpose(pt, wq_sb[g][ob], identr)
                # cast-copy lands this ob's W_eff^T column block as bf16
                nc.scalar.copy(wg[:, ob * 128 : (ob + 1) * 128], pt.bitcast(F32))
            del wq_sb[g]

        def emit_mm0(g):
            # chunk 0 consumes finished groups two PE rounds behind
            for mb in range(NOB):
                nc.tensor.matmul(
                    chains0[mb],
                    lhsT=xt0[:, g, mb * 128 : (mb + 1) * 128],
                    rhs=weff[g],
                    start=(g == 0),
                    stop=(g == NG - 1),
                    skip_group_check=True,
                )

        for g in range(NG):
            gq, gi = divmod(g, 4)
            if gi == 0 and gq + 2 < NGQ:
                wq_tiles[gq + 2] = load_wquad(gq + 2)
            d_ps = emit_lora(g)
            emit_elemwise(g, d_ps)
            if g >= 2:
                emit_mm0(g - 2)
            if g >= 1:
                emit_transpose(g - 1)
        emit_mm0(NG - 2)
        emit_transpose(NG - 1)
        emit_mm0(NG - 1)

        for mb in range(NOB):
            o_sb = opool.tile([128, OS], F32, tag="o")
            nc.scalar.copy(o_sb, chains0[mb])
            nc.sync.dma_start(out=out_d[mb * 128 : (mb + 1) * 128, :], in_=o_sb)

        # ---- phase C: stream remaining x chunks through the big matmul ----
        xts = {0: xt0, 1: xt1}
        for msb in range(1, NMSB):
            if msb + 1 < NMSB:
                xts[msb + 1] = load_chunk(msb + 1)
            xt_t = xts.pop(msb)
            for mb in range(NOB):
                mbg = msb * NOB + mb
                p_out = psumO.tile([128, OS], F32, tag=f"o{mb}")
                for g in range(NG):
                    nc.tensor.matmul(
                        p_out,
                        lhsT=xt_t[:, g, mb * 128 : (mb + 1) * 128],
                        rhs=weff[g],
                        start=(g == 0),
                        stop=(g == NG - 1),
                    )
                o_sb = opool.tile([128, OS], F32, tag="o")
                nc.scalar.copy(o_sb, p_out)
                nc.sync.dma_start(
                    out=out_d[mbg * 128 : (mbg + 1) * 128, :], in_=o_sb
                )

    nc.compile()
    return nc


def _get_program(cb0, tk, dk, reps=1):
    key = (round(float(cb0), 9), tuple(round(float(t), 9) for t in tk),
           tuple(round(float(d), 9) for d in dk), reps)
    if key not in _cache:
        _cache[key] = _build_program(cb0, tk, dk, reps)
    return _cache[key]


def _codebook_consts(codebook):
    cb = np.asarray(codebook, dtype=np.float64)
    cb = cb / max(float(np.max(np.abs(cb))), 1e-8)
    tk = (cb[:-1] + cb[1:]) * 0.5
    dk = np.diff(cb)
    return float(cb[0]), [float(v) for v in tk], [float(v) for v in dk]


def _prep_in_maps(x, weight, scale_log, lora_A, lora_B):
    xf = np.ascontiguousarray(x.reshape(M, I), dtype=np.float32)
    # pack x^T chunks: xt[msb*128 + p, g, m'] = x[msb*MSB + m', g*128 + p]
    xt = (
        xf.reshape(NMSB, MSB, NG, GROUP)
        .transpose(0, 3, 2, 1)
        .astype(ml_dtypes.bfloat16)
        .reshape(NMSB * 128, NG, MSB)
    )
    in_maps = []
    for c in range(N_CORES):
        sl = slice(c * OS, (c + 1) * OS)
        in_maps.append({
            "xt": xt,
            "w": np.ascontiguousarray(
                weight[sl], dtype=np.float32).reshape(OS, NG, GROUP),
            "scl": np.exp(np.ascontiguousarray(
                scale_log.reshape(O, NG)[sl], dtype=np.float32)),
            "la": np.ascontiguousarray(lora_A, dtype=np.float32),
            "lbt": np.ascontiguousarray(lora_B[sl].T, dtype=np.float32),
        })
    return in_maps


def kernel(x, weight, scale_log, codebook, lora_A, lora_B):
    cb0, tk, dk = _codebook_consts(codebook)
    nc = _get_program(cb0, tk, dk)
    in_maps = _prep_in_maps(x, weight, scale_log, lora_A, lora_B)
    res = run_bass_kernel_spmd(nc, in_maps, core_ids=list(range(N_CORES))).results
    out = np.concatenate([res[c]["out"] for c in range(N_CORES)], axis=1)
    return out.reshape(x.shape[0], x.shape[1], O)
